# revision 30
# baseline (speedup 1.0000x reference)
"""Trainium2 Bass kernel for CustomMultiheadAttention.

Full MHA: L=S=2048, N=2, E=1024, H=16, D=64, fp32, returns
(attn_output [L,N,E], attn_weights [N,L,S] = mean over heads).

Sharding (8 cores): core c owns batch n=c//4 and query rows
[ (c%4)*512, (c%4)*512+512 ) with ALL heads local, so softmax, the
head-mean of attention weights, and the output projection are all
core-local; no cross-core reduction is needed.

Per-core pipeline (all matmuls keep a 512-wide moving dim; q/k path in
float32r = tf32 at full PE rate; softmax tail in fp16 for the DVE
2x_1P mode, 8x more precise than bf16):
  B) v = value @ Wv.T          -> SBUF [S,16,65] fp16 (ones col = 1.0;
     bv is folded in after the AV normalize where it is per-partition)
  C) kT = (key @ Wk.T + bk).T  -> SBUF [hd, S] f32r
  D) qT = (query @ Wq.T + bq).T-> SBUF [hd, 512] f32r
  E) per head: scoresT[s,l] (K=64 f32r) -> et = exp(s/8 - 6) fp16 ->
     AV matmul (ones col gives denom = sum(et)) -> PE outer-product
     broadcasts denom -> rb = 1/denom (f32 + fp16 copy) ->
     outT = AV*rb + bv ; w_acc += et*rb (fp16, = sum_h attn_h)
  F) out_part = outT.T @ Wo.T  -> DRAM [512, 1024] f32 (bo on host)
Outputs: out_part f32; w_partT [S, 512] fp16 (host: .T/16 -> f32).
"""

import sys

for _p in ("/opt/trn_rl_repo",):
    if _p not in sys.path:
        sys.path.insert(0, _p)

from contextlib import ExitStack

import numpy as np

import concourse.bass as bass
from concourse import bacc
import concourse.mybir as mybir
import concourse.tile as tile

E = 1024
H = 16
D = 64
S = 2048
N = 2
L = 2048
LQ = 512          # query rows per core
NCORES = 8
F32 = mybir.dt.float32
F32R = mybir.dt.float32r
F16 = mybir.dt.float16
MULT = mybir.AluOpType.mult
EXP_BIAS = -6.0   # keeps 1/denom inside fp16 normal range

_NC_CACHE = None


def build_nc(loop=1):
    nc = bacc.Bacc()

    # ---- DRAM I/O (per core) ----
    qT_in = nc.dram_tensor("qT_in", [E, LQ], F32R, kind="ExternalInput")
    kT_in = nc.dram_tensor("kT_in", [E, S], F32R, kind="ExternalInput")
    vT_in = nc.dram_tensor("vT_in", [E, S], F16, kind="ExternalInput")
    wqT_in = nc.dram_tensor("wqT_in", [E, E], F32R, kind="ExternalInput")
    wkT_in = nc.dram_tensor("wkT_in", [E, E], F32R, kind="ExternalInput")
    wvT_in = nc.dram_tensor("wvT_in", [E, E], F16, kind="ExternalInput")
    woT_in = nc.dram_tensor("woT_in", [E, E], F32R, kind="ExternalInput")
    bq_in = nc.dram_tensor("bq_in", [E], F32, kind="ExternalInput")
    bk_in = nc.dram_tensor("bk_in", [E], F32, kind="ExternalInput")
    bv_in = nc.dram_tensor("bv_in", [E], F32, kind="ExternalInput")
    out_part = nc.dram_tensor("out_part", [LQ, E], F32, kind="ExternalOutput")
    w_partT = nc.dram_tensor("w_partT", [S, LQ], F16, kind="ExternalOutput")

    with tile.TileContext(nc) as tc, ExitStack() as ctx:
        big = ctx.enter_context(tc.tile_pool(name="big", bufs=1))
        persist = ctx.enter_context(tc.tile_pool(name="persist", bufs=1))
        wch = ctx.enter_context(tc.tile_pool(name="wch", bufs=3))
        rhs_pool = ctx.enter_context(tc.tile_pool(name="rhs", bufs=8))
        vch_pool = ctx.enter_context(tc.tile_pool(name="vch", bufs=2))
        exp_pool = ctx.enter_context(tc.tile_pool(name="expp", bufs=9))
        tmp_pool = ctx.enter_context(tc.tile_pool(name="tmpp", bufs=1))
        rb_pool = ctx.enter_context(tc.tile_pool(name="rbp", bufs=2))
        rb16_pool = ctx.enter_context(tc.tile_pool(name="rb16p", bufs=1))
        row_pool = ctx.enter_context(tc.tile_pool(name="rowp", bufs=1))
        oev_pool = ctx.enter_context(tc.tile_pool(name="oev", bufs=1))
        ps_proj = ctx.enter_context(tc.tile_pool(name="psproj", bufs=3, space="PSUM"))
        ps_sc = ctx.enter_context(tc.tile_pool(name="pssc", bufs=2, space="PSUM"))
        ps_av = ctx.enter_context(tc.tile_pool(name="psav", bufs=2, space="PSUM"))
        ps_bc = ctx.enter_context(tc.tile_pool(name="psbc", bufs=1, space="PSUM"))

        def body(_iv=None):
            # ---- persistent SBUF ----
            v_sb = persist.tile([128, 16, H, D + 1], F16, tag="v_sb")
            qT_sb = persist.tile([128, 8, LQ], F32R, tag="qT_sb")
            outT_sb = persist.tile([128, 8, LQ], F32R, tag="outT_sb")
            w_acc = persist.tile([128, 16, LQ], F16, tag="w_acc")
            bq_sb = persist.tile([128, 8], F32, tag="bq_sb")
            bk_sb = persist.tile([128, 8], F32, tag="bk_sb")
            bv_sb = persist.tile([128, 8], F32, tag="bv_sb")
            c1f = persist.tile([1, 128], F32, tag="c1f")
            ebias = persist.tile([128, 1], F32, tag="ebias")
            c1 = persist.tile([1, 128], F32R, tag="c1")
            kT_sb = big.tile([128, 8, S], F32R, tag="bigslot")

            nc.vector.memset(ebias, EXP_BIAS)
            nc.vector.memset(c1f, 1.0)
            nc.vector.tensor_copy(out=c1, in_=c1f)
            nc.sync.dma_start(out=bq_sb, in_=bq_in.rearrange("(m p) -> p m", p=128))
            nc.sync.dma_start(out=bk_sb, in_=bk_in.rearrange("(m p) -> p m", p=128))
            nc.sync.dma_start(out=bv_sb, in_=bv_in.rearrange("(m p) -> p m", p=128))
            nc.vector.memset(v_sb[:, :, :, D], 1.0)  # denominator ones column

            # ---- key projection, kT[hd, s], for head blocks ms x s-chunks nscs
            def emit_C(ms, nscs, queues=None):
                queues = queues or [nc.sync, nc.gpsimd]
                for nsc in nscs:
                    k_rhs = []
                    for kc in range(8):
                        t = rhs_pool.tile([128, 512], F32R, tag="rhs_s")
                        queues[kc % len(queues)].dma_start(
                            out=t,
                            in_=kT_in[
                                kc * 128 : kc * 128 + 128,
                                nsc * 512 : nsc * 512 + 512,
                            ],
                        )
                        k_rhs.append(t)
                    for m in ms:
                        w = wch.tile([128, 8, 128], F32R, tag="wch")
                        eng = nc.gpsimd if m % 2 == 0 else nc.sync
                        eng.dma_start(
                            out=w,
                            in_=wkT_in[:, m * 128 : m * 128 + 128].rearrange(
                                "(k p) c -> p k c", p=128
                            ),
                        )
                        ps = ps_proj.tile([128, 512], F32, tag="psproj")
                        for kc in range(8):
                            nc.tensor.matmul(
                                out=ps,
                                lhsT=w[:, kc, :],
                                rhs=k_rhs[kc],
                                start=(kc == 0),
                                stop=(kc == 7),
                            )
                        nc.vector.tensor_scalar_add(
                            out=kT_sb[:, m, nsc * 512 : nsc * 512 + 512],
                            in0=ps,
                            scalar1=bk_sb[:, m : m + 1],
                        )

            # ---- query projection, qT[hd, l]
            def emit_D(queues):
                q_rhs = []
                for kc in range(8):
                    t = rhs_pool.tile([128, 512], F32R, tag="rhs_s")
                    queues[kc % len(queues)].dma_start(
                        out=t, in_=qT_in[kc * 128 : kc * 128 + 128, :]
                    )
                    q_rhs.append(t)
                for m in range(8):
                    w = wch.tile([128, 8, 128], F32R, tag="wch")
                    eng = nc.gpsimd if m % 2 == 0 else nc.sync
                    eng.dma_start(
                        out=w,
                        in_=wqT_in[:, m * 128 : m * 128 + 128].rearrange(
                            "(k p) c -> p k c", p=128
                        ),
                    )
                    ps = ps_proj.tile([128, 512], F32, tag="psproj")
                    for kc in range(8):
                        nc.tensor.matmul(
                            out=ps,
                            lhsT=w[:, kc, :],
                            rhs=q_rhs[kc],
                            start=(kc == 0),
                            stop=(kc == 7),
                        )
                    nc.vector.tensor_scalar_add(
                        out=qT_sb[:, m, :], in0=ps, scalar1=bq_sb[:, m : m + 1]
                    )

            # ---- value projection (fp16, streamed lhsT), head half nh
            def emit_B(nh, scs, queues=None):
                queues = queues or [nc.sync, nc.gpsimd]
                wv_t = []
                for kc in range(8):
                    t = rhs_pool.tile([128, 512], F16, tag="rhs_s")
                    queues[kc % len(queues)].dma_start(
                        out=t,
                        in_=wvT_in[
                            kc * 128 : kc * 128 + 128, nh * 512 : nh * 512 + 512
                        ],
                    )
                    wv_t.append(t)
                for sc in scs:
                    vch = vch_pool.tile([128, 8, 128], F16, tag="vch")
                    nc.gpsimd.dma_start(
                        out=vch,
                        in_=vT_in[:, sc * 128 : sc * 128 + 128].rearrange(
                            "(k p) c -> p k c", p=128
                        ),
                    )
                    ps = ps_proj.tile([128, 512], F32, tag="psproj")
                    for kc in range(8):
                        nc.tensor.matmul(
                            out=ps,
                            lhsT=vch[:, kc, :],
                            rhs=wv_t[kc],
                            start=(kc == 0),
                            stop=(kc == 7),
                        )
                    nc.vector.tensor_copy(
                        out=v_sb[:, sc, nh * 8 : nh * 8 + 8, 0:D],
                        in_=ps.rearrange("p (h d) -> p h d", d=D),
                    )

            # ---- attention for one head
            def emit_E(h):
                m = h // 2
                hp = 64 * (h % 2)
                exps = []
                for scp in range(8):
                    e_t = exp_pool.tile([128, 2, 512], F16, tag="expp")
                    exps.append(e_t)
                    for j in range(2):
                        sc = scp * 2 + j
                        ps_s = ps_sc.tile([128, 512], F32, tag="pssc")
                        nc.tensor.matmul(
                            out=ps_s,
                            lhsT=kT_sb[hp : hp + 64, m, sc * 128 : sc * 128 + 128],
                            rhs=qT_sb[hp : hp + 64, m, :],
                            start=True,
                            stop=True,
                        )
                        nc.scalar.activation(
                            out=e_t[:, j, :],
                            in_=ps_s,
                            func=mybir.ActivationFunctionType.Exp,
                            scale=0.125,
                            bias=ebias[:, 0:1],
                        )
                ps_o = ps_av.tile([D + 1, 512], F32, tag="psav")
                for sc in range(16):
                    nc.tensor.matmul(
                        out=ps_o,
                        lhsT=v_sb[:, sc, h, :],
                        rhs=exps[sc // 2][:, sc % 2, :],
                        start=(sc == 0),
                        stop=(sc == 15),
                    )
                # denom row -> PE broadcast -> reciprocal (f32 + fp16)
                drow = row_pool.tile([1, 512], F32R, tag="rowp")
                nc.scalar.activation(
                    out=drow, in_=ps_o[D : D + 1, :],
                    func=mybir.ActivationFunctionType.Copy,
                )
                ps_b = ps_bc.tile([128, 512], F32, tag="psbc")
                nc.tensor.matmul(out=ps_b, lhsT=c1, rhs=drow, start=True, stop=True)
                rb = rb_pool.tile([128, 512], F32, tag="rbp")
                nc.vector.reciprocal(out=rb, in_=ps_b)  # 1/denom
                rb16 = rb16_pool.tile([128, 2, 512], F16, tag="rb16p")
                for j in range(2):
                    nc.scalar.activation(
                        out=rb16[:, j, :], in_=rb,
                        func=mybir.ActivationFunctionType.Copy,
                    )
                # normalized head output (transposed) + bv
                nc.vector.tensor_mul(
                    out=outT_sb[hp : hp + 64, m, :], in0=ps_o[0:D, :], in1=rb[0:64, :]
                )
                nc.vector.tensor_scalar_add(
                    out=outT_sb[hp : hp + 64, m, :],
                    in0=outT_sb[hp : hp + 64, m, :],
                    scalar1=bv_sb[hp : hp + 64, m : m + 1],
                )
                # attention-weight accumulation: w_acc += et * rb (fp16 2x,
                # batched over sc pairs for fewer DVE ops)
                for scp in range(8):
                    wpair = w_acc[:, 2 * scp : 2 * scp + 2, :]
                    if h == 0:
                        nc.vector.tensor_mul(out=wpair, in0=exps[scp], in1=rb16)
                    else:
                        t = tmp_pool.tile([128, 2, 512], F16, tag="tmpp")
                        nc.vector.tensor_mul(out=t, in0=exps[scp], in1=rb16)
                        nc.vector.tensor_add(out=wpair, in0=wpair, in1=t)

            # ---- emission schedule: projections overlapped with attention
            wideq = [nc.sync, nc.gpsimd, nc.scalar]
            emit_C([0, 1, 2, 3], [0, 1, 2, 3], queues=wideq)
            emit_D(wideq)
            emit_B(0, range(16), queues=wideq)
            for h in range(H):
                emit_E(h)
                if h < 4:
                    emit_C([4, 5, 6, 7], [h])
                elif h == 4:
                    emit_B(1, range(8))
                elif h == 5:
                    emit_B(1, range(8, 16))

            # weights out (host divides by H)
            for sc in range(16):
                nc.sync.dma_start(
                    out=w_partT[sc * 128 : sc * 128 + 128, :], in_=w_acc[:, sc, :]
                )

            # ---- output projection (bo on host)
            for ec in range(2):
                o_rhs = []
                for kc in range(8):
                    t = rhs_pool.tile([128, 512], F32R, tag="rhs_s")
                    nc.sync.dma_start(
                        out=t,
                        in_=woT_in[
                            kc * 128 : kc * 128 + 128, ec * 512 : ec * 512 + 512
                        ],
                    )
                    o_rhs.append(t)
                for lb in range(4):
                    ps = ps_proj.tile([128, 512], F32, tag="psproj")
                    for kc in range(8):
                        nc.tensor.matmul(
                            out=ps,
                            lhsT=outT_sb[:, kc, lb * 128 : lb * 128 + 128],
                            rhs=o_rhs[kc],
                            start=(kc == 0),
                            stop=(kc == 7),
                        )
                    oev = oev_pool.tile([128, 512], F32, tag="oev")
                    nc.scalar.activation(
                        out=oev, in_=ps, func=mybir.ActivationFunctionType.Copy
                    )
                    nc.sync.dma_start(
                        out=out_part[
                            lb * 128 : lb * 128 + 128, ec * 512 : ec * 512 + 512
                        ],
                        in_=oev,
                    )

        if loop > 1:
            with tc.For_i(0, loop, 1):
                body()
        else:
            body()
    nc.compile()
    return nc


def _tf32_round(x):
    """Round fp32 to the tf32 grid (10-bit mantissa) like the PE consumes."""
    u = np.ascontiguousarray(x, np.float32).view(np.uint32)
    u = (u + 0x1000 + ((u >> 13) & 1)) & np.uint32(0xFFFFE000)
    return u.view(np.float32)


def prep_inputs(query, key, value, Wq, bq, Wk, bk, Wv, bv, Wo, bo):
    """Build the 8 per-core input maps (host-side shard/transpose)."""
    shared = {
        "wqT_in": _tf32_round(Wq.T),
        "wkT_in": _tf32_round(Wk.T),
        "wvT_in": np.ascontiguousarray(Wv.T).astype(np.float16),
        "woT_in": _tf32_round(Wo.T),
        "bq_in": np.ascontiguousarray(bq.astype(np.float32)),
        "bk_in": np.ascontiguousarray(bk.astype(np.float32)),
        "bv_in": np.ascontiguousarray(bv.astype(np.float32)),
    }
    per_n = {}
    for n in range(N):
        per_n[n] = {
            "kT_in": _tf32_round(key[:, n, :].T),
            "vT_in": np.ascontiguousarray(value[:, n, :].T).astype(np.float16),
        }
    in_maps = []
    for c in range(NCORES):
        n = c // 4
        l0 = (c % 4) * LQ
        m = dict(shared)
        m.update(per_n[n])
        m["qT_in"] = _tf32_round(query[l0 : l0 + LQ, n, :].T)
        in_maps.append(m)
    return in_maps


def assemble(results, bo):
    attn_output = np.empty((L, N, E), np.float32)
    attn_weights = np.empty((N, L, S), np.float32)
    for c in range(NCORES):
        n = c // 4
        l0 = (c % 4) * LQ
        attn_output[l0 : l0 + LQ, n, :] = results[c]["out_part"] + bo[None, :]
        attn_weights[n, l0 : l0 + LQ, :] = (
            results[c]["w_partT"].astype(np.float32).T * (1.0 / H)
        )
    return attn_output, attn_weights


def kernel(query, key, value, Wq, bq, Wk, bk, Wv, bv, Wo, bo):
    global _NC_CACHE
    from concourse.bass_utils import run_bass_kernel_spmd

    if _NC_CACHE is None:
        _NC_CACHE = build_nc()
    in_maps = prep_inputs(
        np.asarray(query), np.asarray(key), np.asarray(value),
        np.asarray(Wq), np.asarray(bq), np.asarray(Wk), np.asarray(bk),
        np.asarray(Wv), np.asarray(bv), np.asarray(Wo), np.asarray(bo),
    )
    res = run_bass_kernel_spmd(_NC_CACHE, in_maps, list(range(NCORES))).results
    return assemble(res, np.asarray(bo, np.float32))
